# revision 51
# baseline (speedup 1.0000x reference)
"""MixLoRA sparse-MoE Trainium2 kernel.

Strategy: tensor-parallel over d_ff (F=4096 -> 512 per core) on 8 NeuronCores.
Every core processes all 1024 tokens for its F-slice; the down-projection
produces per-core partial sums over its F-slice which are reduced on the host.

Device layout is feature-major ("transposed"): activations are [feat, token]
so every matmul contraction axis lands on SBUF partitions with zero on-device
transposes.

Precision split (validated vs the fp32 reference in numpy emulation):
  * router + LoRA-A run in float32r on an fp32 copy of x -> top-2 routing
    decisions are exact, so no routing flips.
  * base gate/up/down weights, LoRA-B/A2/B2, activations and outputs are
    bf16 (fp32 PSUM accumulation): max-rel-err ~5e-3, well inside 2e-2.

Schedule:
  * All DMAs ride the SP (sync) queue (no compute queue ever stalls behind
    a dma_start's HWDGE slot); issue order == consumption order.
  * One long-lived 8-bank PSUM pool with manual tag assignment -- pool
    scopes act as barriers, so bank reuse is expressed per-bank (WAR on
    the prior reader only), never per-pool.
  * Phase 1: bf16 x + packed gate/up weights stream in chunk-paced; the 8
    base gate/up psum chains for token-tile 0 follow chunk arrival,
    hiding the fp32 router-x load entirely.
  * Phase 2: per-token-tile router + LoRA-A f32r chains with the top-2
    mask chain (DVE/Pool) and mask-replicate matmuls interleaved; the 4
    routing-weight sigmoids run as one batch so the activation table only
    swaps Silu->Sigmoid->Silu once.
  * Phases 3-5: per f-tile SwiGLU units. Base values are pre-loaded into
    the delta PSUM banks (engine copy + matmul start=False) so every
    branch gets base+delta fused in PSUM with no standalone adds. Down
    chains emit paired [2,NT] output tiles -> 8 output DMAs.
"""
import sys

sys.path.insert(0, "/opt/trn_rl_repo")

from contextlib import ExitStack

import numpy as np
import ml_dtypes

import concourse.tile as tile
from concourse import bacc, bass_isa, mybir
from concourse.bass_utils import run_bass_kernel_spmd

f32 = mybir.dt.float32
f32r = mybir.dt.float32r
bf16 = mybir.dt.bfloat16
AF = mybir.ActivationFunctionType
ALU = mybir.AluOpType
RED = bass_isa.ReduceOp

NCORES = 8
N = 1024          # tokens (B*S)
D = 1024          # hidden
F = 4096          # d_ff
FC = F // NCORES  # 512 per-core f-slice
E = 8             # experts
R = 16            # lora rank
ER = E * R        # 128
NT = 512          # token tile (free dim of matmuls)
P = 128
DT = D // P       # 8
FT = FC // P      # 4
TT = N // NT      # 2

_CACHE = {}


def _build(reps=1):
    nc = bacc.Bacc("TRN2", target_bir_lowering=False, debug=False)

    xbf_d = nc.dram_tensor("xbf", [D, N], bf16, kind="ExternalInput")
    w13_d = nc.dram_tensor("w13", [D, 2 * FC], bf16, kind="ExternalInput")
    xlo_d = nc.dram_tensor("xlo", [D, N], bf16, kind="ExternalInput")
    ghilo_d = nc.dram_tensor("ghilo", [D, 40], bf16, kind="ExternalInput")
    a13_d = nc.dram_tensor("a13", [D, 2 * ER], bf16, kind="ExternalInput")
    b1t_d = nc.dram_tensor("b1t", [ER, FC], bf16, kind="ExternalInput")
    b3a2_d = nc.dram_tensor("b3a2", [ER, 2 * FC], bf16, kind="ExternalInput")
    wdt_d = nc.dram_tensor("wdt", [FC, D], bf16, kind="ExternalInput")
    b2f_d = nc.dram_tensor("b2f", [ER, D], bf16, kind="ExternalInput")
    outT_d = nc.dram_tensor("outT", [D, N], bf16, kind="ExternalOutput")

    r16_np = np.zeros((E, ER), dtype=np.float32)
    for e in range(E):
        r16_np[e, e * R:(e + 1) * R] = 1.0
    r16_d = nc.inline_tensor(r16_np, name="r16")

    with tile.TileContext(nc) as tc:
      for rep in range(reps):
       with ExitStack() as ctx:
        sb = ctx.enter_context(tc.tile_pool(name=f"sb{rep}", bufs=1))
        psm = ctx.enter_context(
            tc.tile_pool(name=f"psm{rep}", bufs=1, space="PSUM"))
        pcnt = [0]

        def pbank(name):
            t = psm.tile([P, NT], f32, tag=f"P{pcnt[0] % 8}", name=name)
            pcnt[0] += 1
            return t

        # ---- persistent SBUF tiles ----
        xbf = sb.tile([P, DT, N], bf16, tag="xbf")
        w13 = sb.tile([P, DT, 2 * FC], bf16, tag="w13")
        xlo = sb.tile([P, DT, N], bf16, tag="xlo")
        gw = sb.tile([P, DT, 40], bf16)
        a13 = sb.tile([P, DT, 2 * ER], bf16, tag="a13")
        a1 = a13[:, :, 0:ER]
        a3 = a13[:, :, ER:2 * ER]
        b1 = sb.tile([ER, FC], bf16)
        b3a2 = sb.tile([ER, 2, FT, P], bf16, tag="b3a2")
        b3 = b3a2[:, 0, :, :]
        wd = sb.tile([P, FT, D], bf16, tag="wd")
        a2 = b3a2[:, 1, :, :]
        b2 = sb.tile([ER, D], bf16)
        r16 = sb.tile([E, ER], f32r)
        c1s0 = sb.tile([P, FT, NT], bf16, tag="c1s0")   # tt0 base gate
        c3s0 = sb.tile([P, FT, NT], bf16, tag="c3s0")   # tt0 base up
        mka = sb.tile([ER, N], f32)
        mkb = sb.tile([ER, N], f32)
        wa_bc = sb.tile([P, N], bf16)
        wb_bc = sb.tile([P, N], bf16)
        m1aT = sb.tile([ER, N], bf16)
        m1bT = sb.tile([ER, N], bf16)
        m3aT = sb.tile([ER, N], bf16)
        m3bT = sb.tile([ER, N], bf16)
        actCT = sb.tile([P, FT, N], bf16, tag="actCT")
        zc = sb.tile([ER, N], bf16)
        logitsT = sb.tile([E, N], f32)

        # ---- DMA issue: single SP queue, need-ordered ----
        def rearr(d, dtype=None):
            src = d[:, :].rearrange("(a p) w -> p a w", p=P)
            if dtype is not None:
                src = src.bitcast(dtype)
            return src

        xbf_src = rearr(xbf_d)
        w13_src = rearr(w13_d)
        wd_src = rearr(wdt_d)
        Q = nc.sync
        Q.dma_start(out=xbf[:, 0, :], in_=xbf_src[:, 0, :])
        Q.dma_start(out=w13[:, 0, :], in_=w13_src[:, 0, :])
        Q.dma_start(out=xbf[:, 1, :], in_=xbf_src[:, 1, :])
        Q.dma_start(out=w13[:, 1, :], in_=w13_src[:, 1, :])
        for i in range(2, DT, 2):
            Q.dma_start(out=xbf[:, i:i + 2, :], in_=xbf_src[:, i:i + 2, :])
            Q.dma_start(out=w13[:, i:i + 2, :], in_=w13_src[:, i:i + 2, :])

        # ---- PE warm-up: build the 3us p-state ramp on junk matmuls while
        # the first x/w chunks stream in (source is a zero-memset tile) ----
        wsrc = sb.tile([P, NT], bf16, tag="wsrc")
        nc.gpsimd.memset(wsrc[:, 0:256], 0)
        pwarm = psm.tile([P, NT], f32, tag="P7", name="pwarm")
        for _ in range(15):
            nc.tensor.matmul(out=pwarm[:, 0:256], lhsT=wsrc[:, 0:P],
                             rhs=wsrc[:, 0:256], start=True, stop=True)

        ts0 = slice(0, NT)
        ts1 = slice(NT, N)
        tsl_of = {0: ts0, 1: ts1}

        # ======== phase 1: base gate/up for token-tile 0, chunk-paced =====
        pX = [pbank(f"pX{ft}") for ft in range(FT)]
        pY = [pbank(f"pY{ft}") for ft in range(FT)]
        for dt_ in range(DT):
            for ft in range(FT):
                nc.tensor.matmul(
                    out=pX[ft][:], lhsT=w13[:, dt_, ft * P:(ft + 1) * P],
                    rhs=xbf[:, dt_, ts0],
                    start=(dt_ == 0), stop=(dt_ == DT - 1))
                nc.tensor.matmul(
                    out=pY[ft][:],
                    lhsT=w13[:, dt_, FC + ft * P:FC + (ft + 1) * P],
                    rhs=xbf[:, dt_, ts0],
                    start=(dt_ == 0), stop=(dt_ == DT - 1))
        for ft in range(FT):
            nc.scalar.copy(out=c1s0[:, ft, :], in_=pX[ft][:])
            nc.vector.tensor_copy(out=c3s0[:, ft, :], in_=pY[ft][:])

        # ======== phase 2: router (f32r, exact) + LoRA-A (bf16) ========
        Q.dma_start(out=a13[:], in_=rearr(a13_d))
        Q.dma_start(out=gw[:], in_=rearr(ghilo_d))
        Q.dma_start(out=r16[:], in_=r16_d[:, :].bitcast(f32r))


        def bank(i, name):
            return psm.tile([P, NT], f32, tag=f"P{i}", name=name)

        with tc.tile_pool(name=f"rs{rep}", bufs=1) as rs:
            m1 = rs.tile([E, N], f32)
            eq1 = rs.tile([E, N], f32r)
            l2 = rs.tile([E, N], f32)
            m2 = rs.tile([E, N], f32)
            eq2 = rs.tile([E, N], f32r)
            diff = rs.tile([1, N], f32)
            wa = rs.tile([1, N], bf16)
            wb = rs.tile([1, N], bf16)

            def mask_chain(tt, branch):
                """DVE/Pool mask math for one token tile and branch."""
                tsl = tsl_of[tt]
                if branch == 0:
                    nc.gpsimd.partition_all_reduce(
                        m1[:, tsl], logitsT[:, tsl], channels=E,
                        reduce_op=RED.max)
                    nc.vector.tensor_tensor(
                        out=eq1[:, tsl], in0=logitsT[:, tsl],
                        in1=m1[:, tsl], op=ALU.is_equal)
                else:
                    nc.vector.scalar_tensor_tensor(
                        out=l2[:, tsl], in0=eq1[:, tsl].bitcast(f32),
                        scalar=-1e30, in1=logitsT[:, tsl],
                        op0=ALU.mult, op1=ALU.add)
                    nc.gpsimd.partition_all_reduce(
                        m2[:, tsl], l2[:, tsl], channels=E,
                        reduce_op=RED.max)
                    nc.vector.tensor_tensor(
                        out=eq2[:, tsl], in0=l2[:, tsl],
                        in1=m2[:, tsl], op=ALU.is_equal)
                    nc.vector.tensor_tensor(
                        out=diff[:, tsl], in0=m2[0:1, tsl],
                        in1=m1[0:1, tsl], op=ALU.subtract)

            def mask_mm(pm, tt, branch):
                tsl = tsl_of[tt]
                eq = eq1 if branch == 0 else eq2
                mk = mka if branch == 0 else mkb
                nc.tensor.matmul(out=pm[:], lhsT=r16[:], rhs=eq[:, tsl],
                                 start=True, stop=True)
                nc.scalar.copy(out=mk[:, tsl], in_=pm[:])

            # Router chains + top-2 mask pipeline first (critical path);
            # LoRA-A chains after them act as xf-tail fillers.
            plg = {0: bank(0, "plg0"), 1: bank(4, "plg1")}
            ps1 = {0: bank(1, "ps1_0"), 1: bank(5, "ps1_1")}
            ps3 = {0: bank(2, "ps3_0"), 1: bank(6, "ps3_1")}
            xlo_src = rearr(xlo_d)
            for i in range(0, DT, 2):
                Q.dma_start(out=xlo[:, i:i + 2, :], in_=xlo_src[:, i:i + 2, :])
            def router_chain(tt):
                """hi-part in psum partitions 0:8, lo-part at 32:40 (engine
                partition bases must be 32-aligned)."""
                tsl = tsl_of[tt]
                for dt_ in range(DT):
                    nc.tensor.matmul(out=plg[tt][0:40, :],
                                     lhsT=gw[:, dt_, :],
                                     rhs=xbf[:, dt_, tsl],
                                     start=(dt_ == 0), stop=(dt_ == DT - 1))
                for dt_ in range(DT):
                    nc.tensor.matmul(out=plg[tt][0:E, :],
                                     lhsT=gw[:, dt_, 0:E],
                                     rhs=xlo[:, dt_, tsl],
                                     start=False, stop=(dt_ == DT - 1),
                                     skip_group_check=True)
                llo = rs.tile([E, N], f32, tag="llo", name="llo")
                nc.scalar.copy(out=llo[:, tsl], in_=plg[tt][32:32 + E, :])
                nc.vector.tensor_tensor(out=logitsT[:, tsl],
                                        in0=plg[tt][0:E, :],
                                        in1=llo[:, tsl], op=ALU.add)

            router_chain(0)
            mask_chain(0, 0)
            router_chain(1)
            pm_a = bank(3, "pm_a")
            mask_mm(pm_a, 0, 0)
            mask_chain(0, 1)
            mask_chain(1, 0)
            pm_b = bank(7, "pm_b")
            mask_mm(pm_b, 0, 1)
            pm_c = bank(3, "pm_c")
            mask_mm(pm_c, 1, 0)
            mask_chain(1, 1)
            pm_d = bank(7, "pm_d")
            mask_mm(pm_d, 1, 1)
            # LoRA-A chains with masked activations interleaved
            for tt in range(TT):
                tsl = tsl_of[tt]
                for dt_ in range(DT):
                    nc.tensor.matmul(out=ps1[tt][:], lhsT=a1[:, dt_, :],
                                     rhs=xbf[:, dt_, tsl],
                                     start=(dt_ == 0), stop=(dt_ == DT - 1))
                nc.vector.tensor_tensor(out=m1aT[:, tsl], in0=ps1[tt][:],
                                        in1=mka[:, tsl], op=ALU.mult)
                nc.vector.tensor_tensor(out=m1bT[:, tsl], in0=ps1[tt][:],
                                        in1=mkb[:, tsl], op=ALU.mult)
            for tt in range(TT):
                tsl = tsl_of[tt]
                for dt_ in range(DT):
                    nc.tensor.matmul(out=ps3[tt][:], lhsT=a3[:, dt_, :],
                                     rhs=xbf[:, dt_, tsl],
                                     start=(dt_ == 0), stop=(dt_ == DT - 1))
                nc.vector.tensor_tensor(out=m3aT[:, tsl], in0=ps3[tt][:],
                                        in1=mka[:, tsl], op=ALU.mult)
                nc.vector.tensor_tensor(out=m3bT[:, tsl], in0=ps3[tt][:],
                                        in1=mkb[:, tsl], op=ALU.mult)

            # batched routing-weight sigmoids: one Silu->Sigmoid->Silu swap
            for tt in range(TT):
                tsl = tsl_of[tt]
                nc.scalar.activation(out=wa[:, tsl], in_=diff[:, tsl],
                                     func=AF.Sigmoid, scale=-1.0)
                nc.scalar.activation(out=wb[:, tsl], in_=diff[:, tsl],
                                     func=AF.Sigmoid)
            for tt in range(TT):
                tsl = tsl_of[tt]
                nc.gpsimd.partition_broadcast(wa_bc[:, tsl], wa[:, tsl])
                nc.gpsimd.partition_broadcast(wb_bc[:, tsl], wb[:, tsl])
        pcnt[0] = 16  # resume bank rotation at P6 (pm banks free first)

        # ======== phases 3-5: units + z + down =========
        Q.dma_start(out=b1[:], in_=b1t_d[:, :])
        Q.dma_start(out=b3a2[:], in_=b3a2_d[:, :])
        Q.dma_start(out=b2[:], in_=b2f_d[:, :])
        Q.dma_start(out=wd[:, 0:2, :], in_=wd_src[:, 0:2, :])
        Q.dma_start(out=wd[:, 2:4, :], in_=wd_src[:, 2:4, :])
        ca_tiles = {}
        cb_tiles = {}
        with tc.tile_pool(name=f"work{rep}", bufs=3) as work, \
                tc.tile_pool(name=f"cpool{rep}", bufs=9) as cpool, \
                tc.tile_pool(name=f"opool{rep}", bufs=4) as opool:

            def branch_math(ft, tt, pA1, pA3, pB1, pB3):
                """Shared SwiGLU/branch elementwise chain; psums hold
                base+delta for all four projections."""
                tsl = tsl_of[tt]
                ua = work.tile([P, NT], bf16, tag="ua")
                nc.scalar.activation(out=ua[:], in_=pA1[:], func=AF.Silu)
                uaw = work.tile([P, NT], bf16, tag="uaw")
                nc.vector.tensor_tensor(out=uaw[:], in0=ua[:],
                                        in1=wa_bc[:, tsl], op=ALU.mult)
                ca = cpool.tile([P, NT], bf16, tag="ca")
                nc.vector.tensor_tensor(out=ca[:], in0=uaw[:], in1=pA3[:],
                                        op=ALU.mult)
                ub = work.tile([P, NT], bf16, tag="ub")
                nc.scalar.activation(out=ub[:], in_=pB1[:], func=AF.Silu)
                ubw = work.tile([P, NT], bf16, tag="ubw")
                nc.vector.tensor_tensor(out=ubw[:], in0=ub[:],
                                        in1=wb_bc[:, tsl], op=ALU.mult)
                cb = cpool.tile([P, NT], bf16, tag="cb")
                nc.vector.tensor_tensor(out=cb[:], in0=ubw[:], in1=pB3[:],
                                        op=ALU.mult)
                ca_tiles[(ft, tt)] = ca
                cb_tiles[(ft, tt)] = cb
                nc.gpsimd.tensor_tensor(out=actCT[:, ft, tsl], in0=ca[:],
                                        in1=cb[:], op=ALU.add)

            def unit_tt0(ft):
                """Base from sbuf; preload psums then accumulate deltas."""
                fsl = slice(ft * P, (ft + 1) * P)
                pA1 = pbank("pA1")
                nc.scalar.copy(out=pA1[:], in_=c1s0[:, ft, :])
                nc.tensor.matmul(out=pA1[:], lhsT=b1[:, fsl],
                                 rhs=m1aT[:, ts0], start=False, stop=True,
                                 skip_group_check=True)
                pA3 = pbank("pA3")
                nc.vector.tensor_copy(out=pA3[:], in_=c3s0[:, ft, :])
                nc.tensor.matmul(out=pA3[:], lhsT=b3[:, ft, :],
                                 rhs=m3aT[:, ts0], start=False, stop=True,
                                 skip_group_check=True)
                pB1 = pbank("pB1")
                nc.scalar.copy(out=pB1[:], in_=c1s0[:, ft, :])
                nc.tensor.matmul(out=pB1[:], lhsT=b1[:, fsl],
                                 rhs=m1bT[:, ts0], start=False, stop=True,
                                 skip_group_check=True)
                pB3 = pbank("pB3")
                nc.vector.tensor_copy(out=pB3[:], in_=c3s0[:, ft, :])
                nc.tensor.matmul(out=pB3[:], lhsT=b3[:, ft, :],
                                 rhs=m3bT[:, ts0], start=False, stop=True,
                                 skip_group_check=True)
                branch_math(ft, 0, pA1, pA3, pB1, pB3)

            def unit_tt1(ft):
                """Fused: base chains in psum; branch-b psums preloaded by
                psum->psum copies; a-deltas accumulate in place."""
                fsl = slice(ft * P, (ft + 1) * P)
                pX_ = pbank("pX_")
                for dt_ in range(DT):
                    nc.tensor.matmul(out=pX_[:],
                                     lhsT=w13[:, dt_, ft * P:(ft + 1) * P],
                                     rhs=xbf[:, dt_, ts1],
                                     start=(dt_ == 0), stop=False)
                pD1 = pbank("pD1")
                nc.scalar.copy(out=pD1[:], in_=pX_[:])
                pY_ = pbank("pY_")
                for dt_ in range(DT):
                    nc.tensor.matmul(
                        out=pY_[:],
                        lhsT=w13[:, dt_, FC + ft * P:FC + (ft + 1) * P],
                        rhs=xbf[:, dt_, ts1],
                        start=(dt_ == 0), stop=False)
                pD3 = pbank("pD3")
                nc.vector.tensor_copy(out=pD3[:], in_=pY_[:])
                nc.tensor.matmul(out=pD1[:], lhsT=b1[:, fsl],
                                 rhs=m1bT[:, ts1], start=False, stop=True,
                                 skip_group_check=True)
                nc.tensor.matmul(out=pD3[:], lhsT=b3[:, ft, :],
                                 rhs=m3bT[:, ts1], start=False, stop=True,
                                 skip_group_check=True)
                # a-branch deltas last: WAR on the psum-preload copies is
                # resolved by then -> no PE stall
                nc.tensor.matmul(out=pX_[:], lhsT=b1[:, fsl],
                                 rhs=m1aT[:, ts1], start=False, stop=True)
                nc.tensor.matmul(out=pY_[:], lhsT=b3[:, ft, :],
                                 rhs=m3aT[:, ts1], start=False, stop=True)
                branch_math(ft, 1, pX_, pY_, pD1, pD3)

            def emit_z(tt):
                tsl = tsl_of[tt]
                pza = pbank("pza")
                for ft in range(FT):
                    nc.tensor.matmul(out=pza[0:ER, :], lhsT=a2[:, ft, :],
                                     rhs=ca_tiles[(ft, tt)][:],
                                     start=(ft == 0), stop=(ft == FT - 1))
                za = cpool.tile([ER, NT], bf16, tag="za")
                nc.vector.tensor_tensor(out=za[:], in0=pza[0:ER, :],
                                        in1=mka[:, tsl], op=ALU.mult)
                pzb = pbank("pzb")
                for ft in range(FT):
                    nc.tensor.matmul(out=pzb[0:ER, :], lhsT=a2[:, ft, :],
                                     rhs=cb_tiles[(ft, tt)][:],
                                     start=(ft == 0), stop=(ft == FT - 1))
                zb = cpool.tile([ER, NT], bf16, tag="zb")
                nc.vector.tensor_tensor(out=zb[:], in0=pzb[0:ER, :],
                                        in1=mkb[:, tsl], op=ALU.mult)
                nc.vector.tensor_tensor(out=zc[:, tsl], in0=za[:], in1=zb[:],
                                        op=ALU.add)

            def down_chain(po, dt_, tsl):
                for ft in range(FT):
                    nc.tensor.matmul(
                        out=po[:],
                        lhsT=wd[:, ft, dt_ * P:(dt_ + 1) * P],
                        rhs=actCT[:, ft, tsl],
                        start=(ft == 0), stop=False)
                nc.tensor.matmul(out=po[:],
                                 lhsT=b2[:, dt_ * P:(dt_ + 1) * P],
                                 rhs=zc[:, tsl], start=False, stop=True)

            def emit_down(tt, dpairs, split_last=False):
                tsl = tsl_of[tt]
                for dpair in dpairs:
                    if split_last and dpair == dpairs[-1]:
                        dt_ = 2 * dpair
                        po = pbank("po")
                        down_chain(po, dt_, tsl)
                        ot1 = opool.tile([P, NT], bf16, tag="os0",
                                         name="ot1_0")
                        nc.scalar.copy(out=ot1[:], in_=po[:])
                        Q.dma_start(out=outT_d[dt_ * P:(dt_ + 1) * P, tsl],
                                    in_=ot1[:])
                        # last output: free-256 halves so copy+DMA of the
                        # first half hides behind the second half's matmuls
                        dt_ = 2 * dpair + 1
                        t0 = tsl.start
                        for h in range(2):
                            hsl = slice(t0 + h * 256, t0 + (h + 1) * 256)
                            po = pbank("po")
                            for ft in range(FT):
                                nc.tensor.matmul(
                                    out=po[:, 0:256],
                                    lhsT=wd[:, ft, dt_ * P:(dt_ + 1) * P],
                                    rhs=actCT[:, ft, hsl],
                                    start=(ft == 0), stop=False)
                            nc.tensor.matmul(out=po[:, 0:256],
                                             lhsT=b2[:, dt_ * P:(dt_ + 1) * P],
                                             rhs=zc[:, hsl],
                                             start=False, stop=True)
                            ot2 = opool.tile([P, 256], bf16, tag=f"oh{h}",
                                             name=f"ot2_{h}")
                            eng = nc.vector if h == 0 else nc.scalar
                            if h == 0:
                                eng.tensor_copy(out=ot2[:], in_=po[:, 0:256])
                            else:
                                eng.copy(out=ot2[:], in_=po[:, 0:256])
                            Q.dma_start(
                                out=outT_d[dt_ * P:(dt_ + 1) * P, hsl],
                                in_=ot2[:])
                        continue
                    ot = opool.tile([P, 2, NT], bf16, tag="ot")
                    for j, dt_ in enumerate((2 * dpair, 2 * dpair + 1)):
                        po = pbank("po")
                        down_chain(po, dt_, tsl)
                        if j == 0:
                            nc.scalar.copy(out=ot[:, j, :], in_=po[:])
                        else:
                            nc.vector.tensor_copy(out=ot[:, j, :], in_=po[:])
                    dsl = slice(2 * dpair * P, (2 * dpair + 2) * P)
                    osrc = outT_d[dsl, tsl].rearrange("(a p) w -> p a w", p=P)
                    Q.dma_start(out=osrc, in_=ot[:])

            for ft in range(FT):
                unit_tt1(ft)
                unit_tt0(ft)
            emit_z(0)
            emit_down(0, range(2))
            emit_z(1)
            emit_down(0, range(2, 4))
            emit_down(1, list(range(4)), split_last=True)
    nc.compile()
    return nc


def _prep_in_maps(inputs):
    hs = np.asarray(inputs["hidden_states"], dtype=np.float32)
    gate_w = np.asarray(inputs["gate_w"], dtype=np.float32)
    w_gate = np.asarray(inputs["w_gate"], dtype=np.float32)
    w_up = np.asarray(inputs["w_up"], dtype=np.float32)
    w_down = np.asarray(inputs["w_down"], dtype=np.float32)
    A1 = np.asarray(inputs["A1"], dtype=np.float32)
    B1 = np.asarray(inputs["B1"], dtype=np.float32)
    A3 = np.asarray(inputs["A3"], dtype=np.float32)
    B3 = np.asarray(inputs["B3"], dtype=np.float32)
    A2 = np.asarray(inputs["A2"], dtype=np.float32)
    B2 = np.asarray(inputs["B2"], dtype=np.float32)

    bf = ml_dtypes.bfloat16
    x = hs.reshape(-1, D)
    C = np.ascontiguousarray
    xT = C(x.T)
    xbf = xT.astype(bf)
    xlo = C(xT - xbf.astype(np.float32)).astype(bf)
    gwT = C(gate_w.T)
    ghi = gwT.astype(bf)
    glo = C(gwT - ghi.astype(np.float32)).astype(bf)
    ghilo_f = np.zeros((D, 40), dtype=np.float32)
    ghilo_f[:, 0:E] = ghi.astype(np.float32)
    ghilo_f[:, 32:32 + E] = glo.astype(np.float32)
    ghilo = C(ghilo_f).astype(bf)
    a13 = C(np.hstack([A1.reshape(ER, D).T,
                       A3.reshape(ER, D).T])).astype(bf)
    b2f = C((2.0 * B2).transpose(0, 2, 1).reshape(ER, D)).astype(bf)

    in_maps = []
    for c in range(NCORES):
        fsl = slice(c * FC, (c + 1) * FC)
        w13 = np.hstack([w_gate[fsl].T, w_up[fsl].T])
        a2t = A2[:, :, fsl].reshape(ER, FC).T          # [FC, ER]
        a2pk = a2t.reshape(FT, P, ER).transpose(1, 0, 2).reshape(P, FC)
        b3t = (2.0 * B3[:, fsl, :]).transpose(0, 2, 1).reshape(ER, FC)
        in_maps.append({
            "xbf": xbf,
            "xlo": xlo,
            "ghilo": ghilo,
            "a13": a13,
            "w13": C(w13).astype(bf),
            "wdt": C(w_down[:, fsl].T).astype(bf),
            "b1t": C((2.0 * B1[:, fsl, :]).transpose(0, 2, 1)
                     .reshape(ER, FC)).astype(bf),
            "b3a2": C(np.hstack([b3t, a2pk])).astype(bf),
            "b2f": b2f,
        })
    return in_maps, hs.shape


def kernel(**inputs):
    if "nc" not in _CACHE:
        _CACHE["nc"] = _build()
    nc = _CACHE["nc"]
    in_maps, (B, S, _) = _prep_in_maps(inputs)
    res = run_bass_kernel_spmd(nc, in_maps, list(range(NCORES)))
    acc = np.zeros((D, N), dtype=np.float64)
    for c in range(NCORES):
        acc += res.results[c]["outT"].astype(np.float32)
    return np.ascontiguousarray(acc.T).astype(np.float32).reshape(B, S, D)
